# revision 33
# baseline (speedup 1.0000x reference)
"""Causal self-attention (separate heads) TRN2 Bass kernel — bf16 rewrite.

Problem (hardcoded): B=4, T=2048, C=1024, H=16, HS=64, fp32 in/out.
  q/k/v = per-head linear projections of x; att = softmax(causal(q k^T / 8));
  y = att v; out = concat_heads(y) @ Wp.T + bp.

Sharding over 8 NeuronCores: core c -> batch b = c//2, head-group hg = c%2
(8 heads each = 4 pairs of heads). Each core computes a [T, C] partial of the
output (its heads' contribution through the column slice of Wp); host sums the
two partials per batch and adds bp.

Design (vs the 462us fp32r baseline; measured ~310us on hw):
  - all matmul operands bf16 (no fp32r small-free-dim penalty, FWL weight
    loads, half DMA + SBUF footprint); PSUM stays fp32; output partials bf16.
  - x^T and all weights SBUF-resident, loaded once; input DMAs split across
    the sync (hardware) and gpsimd (software) DGE queues in parallel; a
    gated warmup matmul burst brings the PE HAM clock to K=8/8 (2.4 GHz)
    right as real compute starts, and a dummy activation pulls the exp
    table load into the DMA window.
  - ONE continuous software pipeline across all 4 head-pair stages: S
    (QK^T) blocks run 3 GI=2 blocks ahead of AV, exp on ACT in between;
    V projection, next-pair QK projection and the C phase are paced filler
    in the PE emission chain (burst-capped so the exp feed never starves).
  - softmax tail: denominator row from ones-row-augmented V; psY banks are
    released early by copying to SBUF; reciprocal via the fast custom-DVE
    op (hw quirk: needs base-partition-0 SBUF input), gpsimd partition
    broadcast, one DVE mul into bf16 YT.  Stage-3 tails run at 128-col
    granularity so C units unlock progressively.
  - PSUM: 5-slot shared 1-bank pool (S blocks + projection groups) + 3
    1-bank psY slots = 8 banks.  QK contraction consumes x chunks in
    DMA-arrival order so the first groups pipeline behind the input stream.
"""
import ml_dtypes
import numpy as np

from concourse import bacc, bass_utils, tile, mybir

B, T, C, H, HS = 4, 2048, 1024, 16, 64
NCORE = 8
NPAIR = 4
NCH = T // 512  # 4 t-chunks of 512
NST = T // 128  # 16 s/t-blocks of 128

f32 = mybir.dt.float32
bf16 = mybir.dt.bfloat16
EXP = mybir.ActivationFunctionType.Exp
LN = mybir.ActivationFunctionType.Ln

_CACHE = {}
DEBUG_DUMPS = False


def _build():
    nc = bacc.Bacc(None, target_bir_lowering=False)

    xT = nc.declare_dram_parameter("xT", [C, T], bf16, isOutput=False)
    wq = nc.declare_dram_parameter("wq", [128, 8, 512], bf16, isOutput=False)
    wk = nc.declare_dram_parameter("wk", [128, 8, 512], bf16, isOutput=False)
    wv = nc.declare_dram_parameter("wv", [128, 8, 512], bf16, isOutput=False)
    wp = nc.declare_dram_parameter("wp", [128, 4, 1024], bf16, isOutput=False)
    bq = nc.declare_dram_parameter("bq", [128, 4], f32, isOutput=False)
    bk = nc.declare_dram_parameter("bk", [128, 4], f32, isOutput=False)
    bv = nc.declare_dram_parameter("bv", [1, 512], bf16, isOutput=False)
    ones = nc.declare_dram_parameter("ones", [1, 128], bf16, isOutput=False)
    trimask = nc.declare_dram_parameter("trimask", [128, 128], bf16, isOutput=False)
    out = nc.declare_dram_parameter("out", [T, C], bf16, isOutput=True)
    if DEBUG_DUMPS:
        d_qt = nc.declare_dram_parameter("d_qt", [128, T], bf16, isOutput=True)
        d_kt = nc.declare_dram_parameter("d_kt", [128, T], bf16, isOutput=True)
        d_v = nc.declare_dram_parameter("d_v", [128, NPAIR, NST, 130], bf16, isOutput=True)
        d_yt = nc.declare_dram_parameter("d_yt", [128, NPAIR, T], bf16, isOutput=True)
        d_es = nc.declare_dram_parameter("d_es", [128, 512], bf16, isOutput=True)
        d_den = nc.declare_dram_parameter("d_den", [1, 1536], f32, isOutput=True)

    with tile.TileContext(nc) as tc:
        with tc.tile_pool(name="persist", bufs=1) as pp:
            # ---- persistent SBUF tensors ----
            xall = pp.tile([128, 8, T], bf16, tag="xall")
            wq_sb = pp.tile([128, 8, 512], bf16, tag="wq_sb")
            wk_sb = pp.tile([128, 8, 512], bf16, tag="wk_sb")
            wv_sb = pp.tile([128, 8, 512], bf16, tag="wv_sb")
            wp_sb = pp.tile([128, 4, 1024], bf16, tag="wp_sb")
            bq_sb = pp.tile([128, 4], f32, tag="bq")
            bk_sb = pp.tile([128, 4], f32, tag="bk")
            bv_sb = pp.tile([1, 512], bf16, tag="bv")
            ones_sb = pp.tile([1, 128], bf16, tag="ones")
            tri_sb = pp.tile([128, 128], bf16, tag="tri")
            bvrep = pp.tile([128, 512], f32, tag="bvrep")
            V = pp.tile([128, NPAIR, NST, 130], bf16, tag="V")
            YT = pp.tile([128, NPAIR, T], bf16, tag="YT")

            # x + wq/wk interleaved (QK(0) needs them first), fine-grained
            # so the 16 DMA queues all pull their weight; wv/wp afterwards.
            for kk in range(3, 8):
                for h in range(2):
                    nc.gpsimd.dma_start(
                        xall[:, kk, 1024 * h : 1024 * h + 1024],
                        xT[128 * kk : 128 * kk + 128, 1024 * h : 1024 * h + 1024],
                    )
            for kk in (3, 4, 5, 6, 7, 0, 1, 2):
                nc.sync.dma_start(wq_sb[:, kk, :], wq[:, kk, :])
                nc.sync.dma_start(wk_sb[:, kk, :], wk[:, kk, :])
            for kk in range(3):
                for h in range(2):
                    nc.sync.dma_start(
                        xall[:, kk, 1024 * h : 1024 * h + 1024],
                        xT[128 * kk : 128 * kk + 128, 1024 * h : 1024 * h + 1024],
                    )
            nc.gpsimd.dma_start(bq_sb[:], bq[:])
            nc.gpsimd.dma_start(bk_sb[:], bk[:])
            nc.gpsimd.dma_start(bv_sb[:], bv[:])
            nc.gpsimd.dma_start(ones_sb[:], ones[:])
            nc.gpsimd.dma_start(tri_sb[:], trimask[:])
            for kk in range(8):
                nc.gpsimd.dma_start(wv_sb[:, kk, :], wv[:, kk, :])
            for p4 in range(4):
                nc.sync.dma_start(wp_sb[:, p4, :], wp[:, p4, :])

            if DEBUG_DUMPS:
                es_dbg = pp.tile([128, 512], bf16, tag="es_dbg")
                den_dbg = pp.tile([1, 1536], f32, tag="den_dbg")

            onescol = pp.tile([128, 32], bf16, tag="onescol")
            nc.vector.memset(onescol[:], 1.0)
            # dummy activation: pulls the exp table load into the DMA window
            actwarm = pp.tile([128, 32], bf16, tag="actwarm")
            nc.scalar.activation(actwarm[:], onescol[:], EXP, scale=0.125)
            for p in range(NPAIR):
                nc.vector.tensor_copy(
                    V[:, p, :, 64:130:65],
                    onescol[:, 0:32].rearrange("s (a b) -> s a b", a=16, b=2),
                )

            # PE emission-order chain, block granular
            _chain = {"prev": None, "first": None}

            def pe_mm(*args, **kw):
                inst = nc.tensor.matmul(*args, **kw)
                if _chain["first"] is None and _chain["prev"] is not None:
                    tile.add_dep_helper(
                        inst.ins, _chain["prev"].ins, sync=False,
                        reason="pe block order",
                    )
                if _chain["first"] is None:
                    _chain["first"] = inst
                _chain["prev"] = inst
                return inst

            def end_blk():
                _chain["first"] = None

            with (
                tc.tile_pool(name="qkt", bufs=2) as pqkt,
                tc.tile_pool(name="es", bufs=8) as pes,
                tc.tile_pool(name="rep", bufs=4) as prep,
                tc.tile_pool(name="ob", bufs=4) as pob,
                tc.tile_pool(name="pss", bufs=2, space="PSUM") as pss,
                tc.tile_pool(name="psy", bufs=4, space="PSUM") as psy,
            ):
                # bvrep = broadcast of bv to 128 partitions (K=1 matmul)
                psb = pss.tile([128, 512], f32, tag="ps", name="ps_bv")
                pe_mm(psb[:], ones_sb[:], bv_sb[:], start=True, stop=True)
                end_blk()
                nc.vector.tensor_copy(bvrep[:], psb[:])

                qt_of = {}
                kt_of = {}

                def alloc_qkt(p):
                    qt_of[p] = pqkt.tile([128, T], bf16, tag="QTp", name="QTp")
                    kt_of[p] = pqkt.tile([128, T], bf16, tag="KTp", name="KTp")

                # ---------- emission units ----------
                def v_unit(st):
                    def go():
                        ps = pss.tile([128, 512], f32, tag="ps", name="ps_v")
                        for kk in range(8):
                            pe_mm(
                                ps[:],
                                xall[:, kk, 128 * st : 128 * st + 128],
                                wv_sb[:, kk, :],
                                start=(kk == 0),
                                stop=(kk == 7),
                            )
                        end_blk()
                        srcv = ps.rearrange("s (p two d) -> s p two d", p=4, two=2)
                        bsrcv = bvrep.rearrange("s (p two d) -> s p two d", p=4, two=2)
                        for hh in range(2):
                            nc.vector.tensor_add(
                                V[:, :, st, 65 * hh : 65 * hh + 64],
                                srcv[:, :, hh, :],
                                bsrcv[:, :, hh, :],
                            )
                    return go

                def qk_unit(p, proj, tch):
                    def go():
                        w_sl = wq_sb if proj == "q" else wk_sb
                        dest = qt_of[p] if proj == "q" else kt_of[p]
                        bias_sb = bq_sb if proj == "q" else bk_sb
                        ps = pss.tile([128, 512], f32, tag="ps", name="ps_qk")
                        order = (3, 4, 5, 6, 7, 0, 1, 2)
                        for ki, kk in enumerate(order):
                            pe_mm(
                                ps[:],
                                w_sl[:, kk, 128 * p : 128 * p + 128],
                                xall[:, kk, 512 * tch : 512 * tch + 512],
                                start=(ki == 0),
                                stop=(ki == 7),
                            )
                        end_blk()
                        nc.vector.tensor_scalar_add(
                            dest[:, 512 * tch : 512 * tch + 512],
                            ps[:],
                            bias_sb[:, p : p + 1],
                        )
                    return go

                def c_unit(m, e):
                    def go():
                        ps = pss.tile([128, 512], f32, tag="ps", name="ps_c")
                        for p4 in range(NPAIR):
                            pe_mm(
                                ps[:],
                                YT[:, p4, 128 * m : 128 * m + 128],
                                wp_sb[:, p4, 512 * e : 512 * e + 512],
                                start=(p4 == 0),
                                stop=(p4 == 3),
                            )
                        end_blk()
                        ob = pob.tile([128, 512], bf16, tag="ob", name="ob")
                        nc.vector.tensor_copy(ob[:], ps[:])
                        eng = nc.sync if (2 * m + e) % 2 == 0 else nc.gpsimd
                        eng.dma_start(
                            out[128 * m : 128 * m + 128, 512 * e : 512 * e + 512],
                            ob[:],
                        )
                    return go

                def emit_tail(p, j, psY, fine=False):
                    for hh in range(2):
                        # custom-DVE recip reads SBUF only (PSUM-in misbehaves
                        # on hw): stage the den row through SBUF first.
                        denrow = prep.tile([1, 512], f32, tag="denrow", name="denrow")
                        nc.vector.tensor_copy(denrow[:], psY[hh][64:65, :])
                        row = prep.tile([1, 512], f32, tag="row", name="row")
                        nc.vector.reciprocal_approx_fast(row[:], denrow[:])
                        repc = prep.tile([64, 512], f32, tag="repc", name="repc")
                        nc.gpsimd.partition_broadcast(repc[:], row[:])
                        if DEBUG_DUMPS and p == 0 and j == 0 and hh == 0:
                            nc.vector.tensor_copy(den_dbg[:, 0:512], psY[hh][64:65, :])
                            nc.vector.tensor_copy(den_dbg[:, 512:1024], row[:])
                            nc.vector.tensor_copy(den_dbg[:, 1024:1536], repc[0:1, :])
                        nc.vector.tensor_mul(
                            YT[64 * hh : 64 * hh + 64, p, 512 * j : 512 * j + 512],
                            psY[hh][0:64, :],
                            repc[:],
                        )

                if DEBUG_DUMPS:
                    nc.sync.dma_start(d_qt[:], qt_of[0][:])
                    nc.sync.dma_start(d_kt[:], kt_of[0][:])

                # ---------- attention stages ----------
                GI = 2  # i-steps per attention block

                # ---------- one continuous pipeline across all 4 stages ----
                # (the S->exp->AV lag crosses stage boundaries, so the exp
                # feed never drains between pairs)
                g_blocks = []   # (p, j, [i...]) GI-bundled
                stage_base = []
                for sp in range(NPAIR):
                    stage_base.append(len(g_blocks))
                    for j in range(NCH):
                        nst_j = 4 * j + 4
                        for i0 in range(0, nst_j, 2):
                            g_blocks.append((sp, j, [i0, i0 + 1]))
                nblk_stage = 20

                filler = []
                cgate = []
                fill_base = []   # first filler index of each stage
                qk_idx_of_tch = {}
                v_idx = {}
                for sp in range(NPAIR):
                    fill_base.append(len(filler))
                    if sp == 0:
                        for tch in range(1, NCH):
                            qk_idx_of_tch[tch] = len(filler)
                            filler.append(qk_unit(0, "q", tch))
                            filler.append(qk_unit(0, "k", tch))
                            cgate += [None, None]
                            for st in range(4 * (tch - 1), 4 * tch):
                                v_idx[st] = len(filler)
                                filler.append(v_unit(st))
                                cgate.append(None)
                        for st in range(12, NST):
                            v_idx[st] = len(filler)
                            filler.append(v_unit(st))
                            cgate.append(None)
                    if sp + 1 < NPAIR:
                        alloc_qkt(sp + 1)
                        for tch in range(NCH):
                            filler.append(qk_unit(sp + 1, "q", tch))
                            filler.append(qk_unit(sp + 1, "k", tch))
                            cgate += [None, None]
                    if sp == NPAIR - 1:
                        for m in range(NST):
                            for e in range(2):
                                filler.append(c_unit(m, e))
                                cgate.append(m)
                fill_base.append(len(filler))

                def ensure_fill(upto):
                    nonlocal fidx
                    while fidx <= upto:
                        filler[fidx]()
                        fidx += 1

                nblk = len(g_blocks)
                eS_store = {}
                psY_of = {}
                tails_pending = []
                tails_done = set()
                fidx = 0
                for n in range(nblk + 3):
                    # deferred tails (release psY banks)
                    while tails_pending and tails_pending[0][0] <= n:
                        _, tp, tj, tpsY = tails_pending.pop(0)
                        emit_tail(tp, tj, tpsY, fine=tp == NPAIR - 1)
                        for m4 in range(4):
                            tails_done.add((tp, 4 * tj + m4))
                    # AV for block n-3
                    if n >= 3:
                        bp, j, ii = g_blocks[n - 3]
                        if bp == 0 and ii[-1] in v_idx:
                            ensure_fill(v_idx[ii[-1]])
                        if (bp, j) not in psY_of:
                            psY_of[(bp, j)] = [
                                psy.tile([65, 512], f32, tag="psY", name="psY")
                                for _ in range(2)
                            ]
                        psYl = psY_of[(bp, j)]
                        nst_j = 4 * j + 4
                        for i in ii:
                            off = max(0, 128 * i - 512 * j)
                            eSs = eS_store.pop((bp, j, i))
                            for hh in range(2):
                                pe_mm(
                                    psYl[hh][:, off:512],
                                    V[:, bp, i, 65 * hh : 65 * hh + 65],
                                    eSs[hh][:, off:512],
                                    start=(i == 0),
                                    stop=(i == nst_j - 1),
                                )
                        end_blk()
                        if ii[-1] == nst_j - 1:
                            tails_pending.append(
                                (n + 1, bp, j, psY_of.pop((bp, j)))
                            )
                    # paced fillers (burst-capped)
                    sp = min(NPAIR - 1, n // nblk_stage)
                    rel = n - stage_base[sp]
                    nfill_sp = fill_base[sp + 1] - fill_base[sp]
                    want = min(
                        fill_base[sp + 1],
                        fill_base[sp]
                        + ((rel + 1) * nfill_sp) // max(1, nblk_stage),
                    )
                    burst = 0
                    while fidx < want and burst < 2 and (
                        cgate[fidx] is None
                        or (NPAIR - 1, cgate[fidx]) in tails_done
                    ):
                        filler[fidx]()
                        fidx += 1
                        burst += 1
                    # S block n
                    if n < nblk:
                        bp, j, ii = g_blocks[n]
                        if bp == 0 and j in qk_idx_of_tch:
                            ensure_fill(qk_idx_of_tch[j] + 1)
                        for i in ii:
                            off = max(0, 128 * i - 512 * j)
                            eSs = []
                            for hh in range(2):
                                h0 = 64 * hh
                                psS = pss.tile([128, 512], f32, tag="ps", name="psS")
                                pe_mm(
                                    psS[:, off:512],
                                    kt_of[bp][h0 : h0 + 64, 128 * i : 128 * i + 128],
                                    qt_of[bp][
                                        h0 : h0 + 64,
                                        512 * j + off : 512 * j + 512,
                                    ],
                                    start=True,
                                    stop=True,
                                )
                                eS = pes.tile([128, 512], bf16, tag="eS", name="eS")
                                nc.scalar.activation(
                                    eS[:, off:512], psS[:, off:512], EXP,
                                    scale=0.125,
                                )
                                if i >= 4 * j:
                                    nc.vector.tensor_mul(
                                        eS[:, off : off + 128],
                                        eS[:, off : off + 128],
                                        tri_sb[:],
                                    )
                                if (
                                    DEBUG_DUMPS and bp == 0 and j == 0 and i == 0
                                    and hh == 0
                                ):
                                    nc.vector.tensor_copy(es_dbg[:], eS[:])
                                eSs.append(eS)
                            eS_store[(bp, j, i)] = eSs
                        end_blk()
                # flush
                while tails_pending:
                    _, tp, tj, tpsY = tails_pending.pop(0)
                    emit_tail(tp, tj, tpsY, fine=tp == NPAIR - 1)
                    for m4 in range(4):
                        tails_done.add((tp, 4 * tj + m4))
                while fidx < len(filler):
                    filler[fidx]()
                    fidx += 1

                if DEBUG_DUMPS:
                    nc.sync.dma_start(d_qt[:], qt_of[0][:])
                    nc.sync.dma_start(d_kt[:], kt_of[0][:])

                # ---------- attention stages ----------
                GI = 2  # i-steps per attention block

                for stage in range(NPAIR):
                    p = stage

                    # filler list: own-pair QK first (gated incrementally by
                    # the S blocks), then V units (stage 0) / C units (stage 3)
                    filler = []
                    cgate = []   # stage-3: t-chunk whose tails a C unit needs
                    if stage == 0:
                        alloc_qkt(0)
                    qk_idx_of_tch = {}
                    tch0 = 0 if stage == 0 else 1  # tch0 pre-emitted by prev
                    for tch in range(tch0, NCH):
                        qk_idx_of_tch[tch] = len(filler)  # index of the q unit
                        for proj in ("q", "k"):
                            filler.append(qk_unit(p, proj, tch))
                            cgate.append(None)
                    n_qk = len(filler)
                    if stage == 0:
                        v_idx0 = len(filler)
                        filler += [v_unit(st) for st in range(NST)]
                        cgate += [None] * NST
                    if stage == NPAIR - 1:
                        for m in range(NST):
                            for e in range(2):
                                filler.append(c_unit(m, e))
                                cgate.append(m)  # needs this 512-tail

                    def ensure_fill(upto):
                        nonlocal fidx
                        while fidx <= upto:
                            filler[fidx]()
                            fidx += 1

                    blocks = []  # (a = 512-wide t-chunk, i = s-block)
                    for a in range(NCH):
                        for i in range(4 * a + 4):
                            blocks.append((a, i))

                    nfill = len(filler)
                    nblk = len(blocks)
                    LAG = 3

                    eS_store = {}
                    psY_of = {}
                    tails_pending = []
                    tails_done = set()
                    fidx = 0
                    for n in range(nblk + LAG):
                        # deferred tails (free the psY bank for reuse)
                        while tails_pending and tails_pending[0][0] <= n:
                            _, tj, tpsY = tails_pending.pop(0)
                            emit_tail(p, tj, tpsY)
                            tails_done.add(tj)
                        # AV for block n-LAG (deep S->exp->AV lookahead)
                        if n >= LAG:
                            a, i = blocks[n - LAG]
                            if stage == 0:
                                ensure_fill(v_idx0 + i)
                            eS = eS_store.pop((a, i))
                            nst_a = 4 * a + 4
                            if a not in psY_of:
                                psY_of[a] = [
                                    psy.tile([65, 512], f32, tag="psY", name="psY")
                                    for _ in range(2)
                                ]
                            psYl = psY_of[a]
                            off = max(0, 128 * i - 512 * a)
                            for hh in range(2):
                                pe_mm(
                                    psYl[hh][:, off:512],
                                    V[:, p, i, 65 * hh : 65 * hh + 65],
                                    eS[:, 512 * hh + off : 512 * hh + 512],
                                    start=(i == 0),
                                    stop=(i == nst_a - 1),
                                )
                            end_blk()
                            if i == nst_a - 1:
                                tails_pending.append(
                                    (n + 1, a, psY_of.pop(a))
                                )
                        # filler unit(s), front-loaded
                        want = min(nfill, ((n + 1) * nfill) // max(1, int(0.7 * nblk)))
                        burst = 0
                        while fidx < want and burst < 2 and (
                            cgate[fidx] is None or cgate[fidx] in tails_done
                        ):
                            filler[fidx]()
                            fidx += 1
                            burst += 1
                        # S block n: both heads into one 2-bank psS tile,
                        # one exp instruction covers both
                        if n < nblk:
                            a, i = blocks[n]
                            if a in qk_idx_of_tch:
                                ensure_fill(qk_idx_of_tch[a] + 1)
                            off = max(0, 128 * i - 512 * a)
                            psS = pss.tile([128, 1024], f32, tag="ps", name="psS")
                            for hh in range(2):
                                h0 = 64 * hh
                                pe_mm(
                                    psS[:, 512 * hh + off : 512 * hh + 512],
                                    kt_of[p][h0 : h0 + 64, 128 * i : 128 * i + 128],
                                    qt_of[p][
                                        h0 : h0 + 64,
                                        512 * a + off : 512 * a + 512,
                                    ],
                                    start=True,
                                    stop=True,
                                )
                            end_blk()
                            eS = pes.tile([128, 1024], bf16, tag="eS", name="eS")
                            if off == 0:
                                nc.scalar.activation(
                                    eS[:], psS[:], EXP, scale=0.125,
                                )
                            else:
                                # skip the uninitialized [512:512+off] gap
                                for hh in range(2):
                                    nc.scalar.activation(
                                        eS[:, 512 * hh + off : 512 * hh + 512],
                                        psS[:, 512 * hh + off : 512 * hh + 512],
                                        EXP, scale=0.125,
                                    )
                            if i >= 4 * a:
                                for hh in range(2):
                                    nc.vector.tensor_mul(
                                        eS[:, 512 * hh + off : 512 * hh + off + 128],
                                        eS[:, 512 * hh + off : 512 * hh + off + 128],
                                        tri_sb[:],
                                    )
                            if (
                                DEBUG_DUMPS and stage == 0 and a == 0 and i == 0
                            ):
                                nc.vector.tensor_copy(es_dbg[:], eS[:, 0:512])
                            eS_store[(a, i)] = eS
                    # flush leftover fillers / tails
                    while tails_pending:
                        _, tj, tpsY = tails_pending.pop(0)
                        emit_tail(p, tj, tpsY, fine=stage == NPAIR - 1)
                        for m4 in range(4):
                            tails_done.add(4 * tj + m4)
                    while fidx < nfill:
                        filler[fidx]()  # all tails emitted; gates satisfied
                        fidx += 1
                    if stage + 1 < NPAIR:
                        # warm up the next stage: its QK t-chunk 0 runs here
                        # so its first S block doesn't stall at stage entry
                        alloc_qkt(stage + 1)
                        qk_unit(stage + 1, "q", 0)()
                        qk_unit(stage + 1, "k", 0)()

                if DEBUG_DUMPS:
                    nc.sync.dma_start(d_qt[:], qt_of[0][:])
                    nc.sync.dma_start(d_kt[:], kt_of[0][:])
                    nc.sync.dma_start(d_v[:], V[:])
                    nc.sync.dma_start(d_yt[:], YT[:])
                    nc.sync.dma_start(d_es[:], es_dbg[:])
                    nc.sync.dma_start(d_den[:], den_dbg[:])

    nc.compile()
    return nc


def _to_bf16(a):
    return np.ascontiguousarray(a.astype(ml_dtypes.bfloat16))


def _prep_core_inputs(x, Wq, bq, Wk, bk, Wv, bv, Wp, core):
    b, hg = core // 2, core % 2
    h0 = 8 * hg

    def wprep(W):
        A = W[h0 : h0 + 8]
        Bm = np.transpose(A, (2, 0, 1)).reshape(C, 512)
        return _to_bf16(Bm.reshape(8, 128, 512).transpose(1, 0, 2))

    def bprep(bias):
        return np.ascontiguousarray(bias[h0 : h0 + 8].reshape(4, 128).T)

    wp_sl = Wp[:, 512 * hg : 512 * hg + 512]
    wp_prep = _to_bf16(wp_sl.T.reshape(4, 128, 1024).transpose(1, 0, 2))

    return {
        "xT": _to_bf16(x[b].T),
        "wq": wprep(Wq),
        "wk": wprep(Wk),
        "wv": wprep(Wv),
        "wp": wp_prep,
        "bq": bprep(bq),
        "bk": bprep(bk),
        "bv": _to_bf16(bv[h0 : h0 + 8].reshape(1, 512)),
        "ones": np.ones((1, 128), dtype=ml_dtypes.bfloat16),
        "trimask": _to_bf16(np.triu(np.ones((128, 128), np.float32))),
    }


TRACE = False
TRACE_KW = {}


def kernel(x, Wq, bq, Wk, bk, Wv, bv, Wp, bp):
    x = np.asarray(x, np.float32)
    Wq = np.asarray(Wq, np.float32)
    bq = np.asarray(bq, np.float32)
    Wk = np.asarray(Wk, np.float32)
    bk = np.asarray(bk, np.float32)
    Wv = np.asarray(Wv, np.float32)
    bv = np.asarray(bv, np.float32)
    Wp = np.asarray(Wp, np.float32)
    bp = np.asarray(bp, np.float32)

    if "nc" not in _CACHE:
        _CACHE["nc"] = _build()
    nc = _CACHE["nc"]

    in_maps = [
        _prep_core_inputs(x, Wq, bq, Wk, bk, Wv, bv, Wp, core)
        for core in range(NCORE)
    ]
    res = bass_utils.run_bass_kernel_spmd(
        nc, in_maps, list(range(NCORE)), trace=TRACE, **TRACE_KW
    )
    _CACHE["last_result"] = res

    outp = np.empty((B, T, C), np.float32)
    for b in range(B):
        outp[b] = res.results[2 * b]["out"] + res.results[2 * b + 1]["out"] + bp
    return outp
